# revision 10
# baseline (speedup 1.0000x reference)
"""FP8Linear (dynamic per-tensor fp8 quantized linear) on 8 Trainium2 cores.

Single fused launch (v2):
  - Core c owns x rows [c*2048:(c+1)*2048] and w rows [c*1024:(c+1)*1024].
  - On-device amax + scales: w first (it unblocks the AllGather path),
    then x.  Cross-core max via two tiny AllReduce(max) collectives;
    scales computed on device (divide + Newton refinement, ~1ulp of the
    reference's IEEE result; grid mismatches this can cause are far
    inside the 2e-2 gate).  Scales are broadcast to all 128 partitions
    with a K=1 matmul against a ones row.
  - Quantize-then-transpose: stripes are scaled to fp8 on the vector
    engine, then transposed as REGULAR fp8 matmuls against an fp8
    identity (~3x cheaper PE time than fp32 transpose-mode and it keeps
    the HAM clock gate warm).  PSUM batches of 4 tiles are evacuated on
    the scalar(ACT)/gpsimd engines so vector stays free.
  - w^T fp8 is AllGathered in 4 quarter chunks; the serial cc stream is
    ordered [ARed(w), gather q0, ARed(x), q1, q2, q3] so x quantization
    is unblocked right after the first gather chunk.
  - DoubleRow fp8 matmul, 8 k-pairs per PSUM bank, 4 banks per
    (quarter, m-tile) round; one LDWEIGHTS per k-pair feeds 4 matmuls
    (ldweights elision).  Quarter-0 rounds are interleaved with the x
    transpose pipeline so the PE never idles.  Epilogue
    (psum*s_out + bias) -> fp16 on vector.
  - bias is host-permuted into gather order; output columns are
    de-permuted by per-256-column output DMAs.  Host stacks the 8
    row-blocks.
"""
import os
import sys

for _p in ("/opt/trn_rl_repo", "/root/.axon_site/_ro/trn_rl_repo"):
    if _p not in sys.path and os.path.isdir(_p):
        sys.path.append(_p)

import numpy as np

import concourse.bass as bass  # noqa: F401
from concourse import bacc, bass_isa
import concourse.mybir as mybir
import concourse.tile as tile
from concourse.bass_utils import run_bass_kernel_spmd
from concourse.masks import make_identity

F32 = mybir.dt.float32
F16 = mybir.dt.float16
FP8 = mybir.dt.float8e4

N_CORES = 8
M_FULL, K, N_FULL = 16384, 2048, 8192
M_LOC = M_FULL // N_CORES            # 2048 x-rows per core
N_LOC = N_FULL // N_CORES            # 1024 w-rows quantized per core
KSUB = K // 128                      # 16
N_TILE = 512                         # psum free dim
NQ = 4                               # AllGather chunks
QW = N_LOC // NQ                     # 256 w-rows per gather chunk
ELIDE_LDW = True                     # one LDWEIGHTS per k-pair (4 matmuls)

TRACE = False
LAST_EXEC_NS = []

DR = mybir.MatmulPerfMode.DoubleRow
AX = mybir.AxisListType
OP = mybir.AluOpType

# f32 constant: 4 / (448*448); folds the 2x half-scale trick back out.
S_OUT_C = float(np.float32(4.0) / np.float32(200704.0))


def _scale_chain(nc, cp, amax_sb, mul_const):
    """mul_const/amax on [1,1] f32 tiles, Newton-refined to ~1ulp."""
    a = cp.tile([1, 1], F32)
    nc.vector.tensor_scalar_max(a[:], amax_sb[:], 1e-12)
    r0 = cp.tile([1, 1], F32)
    nc.vector.reciprocal(r0[:], a[:])
    # r1 = 2*r0 - r0*(a*r0)
    t1 = cp.tile([1, 1], F32)
    nc.vector.tensor_tensor(t1[:], a[:], r0[:], op=OP.mult)
    t2 = cp.tile([1, 1], F32)
    nc.vector.tensor_tensor(t2[:], r0[:], t1[:], op=OP.mult)
    r1 = cp.tile([1, 1], F32)
    nc.vector.scalar_tensor_tensor(
        out=r1[:], in0=r0[:], scalar=2.0, in1=t2[:],
        op0=OP.mult, op1=OP.subtract)
    # q0 = c*r1 ; q = q0 - r1*(a*q0 - c)
    q0 = cp.tile([1, 1], F32)
    nc.vector.tensor_scalar_mul(q0[:], r1[:], float(mul_const))
    u1 = cp.tile([1, 1], F32)
    nc.vector.tensor_tensor(u1[:], a[:], q0[:], op=OP.mult)
    u2 = cp.tile([1, 1], F32)
    nc.vector.tensor_scalar_sub(u2[:], u1[:], float(mul_const))
    u3 = cp.tile([1, 1], F32)
    nc.vector.tensor_tensor(u3[:], r1[:], u2[:], op=OP.mult)
    q = cp.tile([1, 1], F32)
    nc.vector.tensor_tensor(q[:], q0[:], u3[:], op=OP.subtract)
    return q


def _build_main():
    nc = bacc.Bacc("TRN2", target_bir_lowering=False, debug=False,
                   num_devices=N_CORES)
    xs = nc.dram_tensor("xs", [M_LOC, K], F32, kind="ExternalInput")
    wl = nc.dram_tensor("wl", [N_LOC, K], F32, kind="ExternalInput")
    bias_in = nc.dram_tensor("bias_in", [1, N_FULL], F16, kind="ExternalInput")
    out = nc.dram_tensor("out", [M_LOC, N_FULL], F16, kind="ExternalOutput")

    ar_w_in = nc.dram_tensor("ar_w_in", [1, 1], F32)
    ar_w_out = nc.dram_tensor("ar_w_out", [1, 1], F32, addr_space="Shared")
    ar_x_in = nc.dram_tensor("ar_x_in", [1, 1], F32)
    ar_x_out = nc.dram_tensor("ar_x_out", [1, 1], F32, addr_space="Shared")
    wT_loc = [nc.dram_tensor(f"wT_loc{q}", [K, QW], FP8) for q in range(NQ)]
    wT_all = [nc.dram_tensor(f"wT_all{q}", [N_CORES, K, QW], FP8,
                             addr_space="Shared") for q in range(NQ)]
    wT_loc_v = [t.ap().rearrange("(ko p) n -> p ko n", p=128) for t in wT_loc]
    groups = [list(range(N_CORES))]

    with tile.TileContext(nc) as tc:
        with (
            tc.tile_pool(name="const", bufs=1) as cp,
            tc.tile_pool(name="xres", bufs=1) as xrp,
            tc.tile_pool(name="ep", bufs=4) as epp,
            tc.tile_pool(name="wt", bufs=2) as wtp,
        ):
            # ---- constants ----
            idf = cp.tile([128, 128], F32)
            make_identity(nc, idf[:])
            id8 = cp.tile([128, 128], FP8)
            nc.scalar.copy(id8[:], idf[:])
            ones_row = cp.tile([1, 128], F32)
            nc.gpsimd.memset(ones_row[:], 1.0)
            bias_row = cp.tile([1, N_FULL], F16)
            nc.sync.dma_start(bias_row[:], bias_in[:])
            bias_t = cp.tile([128, N_FULL], F16)
            nc.gpsimd.partition_broadcast(bias_t[:], bias_row[:], channels=128)
            scw = cp.tile([128, 1], F32)
            scx = cp.tile([128, 2], F32)   # [x half-scale, s_out]
            xr = xrp.tile([128, KSUB, M_LOC], FP8)   # x^T fp8 resident

            with (
                tc.tile_pool(name="wstr", bufs=2) as wsp,
                tc.tile_pool(name="xstr", bufs=3) as sp,
                tc.tile_pool(name="q8", bufs=3) as qp,
                tc.tile_pool(name="wa", bufs=1) as wap,
                tc.tile_pool(name="tp", bufs=2, space="PSUM") as tpp,
                tc.tile_pool(name="mm", bufs=1, space="PSUM") as mp,
            ):
                # ---- w stripes in (resident), partial absmax ----
                pmw = cp.tile([128, N_LOC // 128], F32)
                for i in range(N_LOC // 128):
                    s = wsp.tile([128, K], F32, tag="w", name=f"wam{i}")
                    nc.sync.dma_start(s[:], wl[i * 128:(i + 1) * 128, :])
                    nc.vector.tensor_reduce(
                        pmw[:, i:i + 1], s[:], axis=AX.X, op=OP.max,
                        apply_absolute_value=True)

                # ---- w amax -> AllReduce#1 -> scale -> broadcast ----
                pw1 = cp.tile([128, 1], F32)
                nc.vector.tensor_reduce(pw1[:], pmw[:], axis=AX.X, op=OP.max)
                # partition-reduce via PE transpose (PE is idle here)
                pswt = tpp.tile([128, 4, 128], F32, tag="t", name="pswt")
                nc.tensor.matmul(pswt[0:1, 0, :], pw1[:], idf[:],
                                 start=True, stop=True)
                aw = cp.tile([1, 1], F32)
                nc.vector.tensor_reduce(
                    aw[:], pswt[0:1, 0, :], axis=AX.X, op=OP.max)
                nc.sync.dma_start(ar_w_in.ap(), aw[:])
                nc.gpsimd.collective_compute(
                    "AllReduce", OP.max, replica_groups=groups,
                    ins=[ar_w_in.ap().opt()], outs=[ar_w_out.ap().opt()])
                arw = cp.tile([1, 1], F32)
                nc.sync.dma_start(arw[:], ar_w_out.ap())
                hw = _scale_chain(nc, cp, arw, 224.0)
                psb = tpp.tile([128, 4, 128], F32, tag="t")
                nc.tensor.matmul(psb[:, 0, 0:1], ones_row[:], hw[:],
                                 start=True, stop=True)
                nc.scalar.copy(scw[:], psb[:, 0, 0:1])

                wa = wap.tile([128, KSUB, N_LOC], FP8)

                def w_stripe(i):
                    s = wsp.tile([128, K], F32, tag="w", name=f"wq_in{i}")
                    nc.sync.dma_start(s[:], wl[i * 128:(i + 1) * 128, :])
                    wq = qp.tile([128, K], FP8, tag="wq", name=f"wq{i}")
                    nc.vector.tensor_scalar_mul(wq[:], s[:], scw[:, 0:1])
                    for kc4 in range(KSUB // 4):
                        t = tpp.tile([128, 4, 128], F32, tag="t")
                        for j in range(4):
                            kc = kc4 * 4 + j
                            nc.tensor.matmul(
                                t[:, j, :], wq[:, kc * 128:(kc + 1) * 128],
                                id8[:], start=True, stop=True)
                        nc.scalar.copy(
                            wa[:, kc4 * 4:kc4 * 4 + 4, i * 128:(i + 1) * 128],
                            t[:])

                def gather_q(qn):
                    nc.sync.dma_start(
                        wT_loc_v[qn][:], wa[:, :, qn * QW:(qn + 1) * QW])
                    nc.gpsimd.collective_compute(
                        "AllGather", OP.bypass, replica_groups=groups,
                        ins=[wT_loc[qn].ap().opt()],
                        outs=[wT_all[qn].ap().opt()])

                # ---- w stripes 0-1 -> gather q0 ----
                w_stripe(0)
                w_stripe(1)
                gather_q(0)

                # ---- x stripes in, partial absmax (vector: after w path) ----
                pmx = cp.tile([128, M_LOC // 128], F32)
                for i in range(M_LOC // 128):
                    s = sp.tile([128, K], F32, tag="x", name=f"xam{i}")
                    nc.sync.dma_start(s[:], xs[i * 128:(i + 1) * 128, :])
                    nc.vector.tensor_reduce(
                        pmx[:, i:i + 1], s[:], axis=AX.X, op=OP.max,
                        apply_absolute_value=True)

                # ---- x amax -> AllReduce#2 (cc stream: right after q0) ----
                px1 = cp.tile([128, 1], F32)
                nc.vector.tensor_reduce(px1[:], pmx[:], axis=AX.X, op=OP.max)
                axb = cp.tile([128, 1], F32)
                nc.gpsimd.partition_all_reduce(
                    axb[:], px1[:], channels=128,
                    reduce_op=bass_isa.ReduceOp.max)
                nc.sync.dma_start(ar_x_in.ap(), axb[0:1, :])
                nc.gpsimd.collective_compute(
                    "AllReduce", OP.max, replica_groups=groups,
                    ins=[ar_x_in.ap().opt()], outs=[ar_x_out.ap().opt()])
                arx = cp.tile([1, 1], F32)
                nc.sync.dma_start(arx[:], ar_x_out.ap())
                hx = _scale_chain(nc, cp, arx, 224.0)
                arwc = cp.tile([1, 1], F32)
                nc.vector.tensor_scalar_max(arwc[:], arw[:], 1e-12)
                arxc = cp.tile([1, 1], F32)
                nc.vector.tensor_scalar_max(arxc[:], arx[:], 1e-12)
                so0 = cp.tile([1, 1], F32)
                nc.vector.tensor_tensor(so0[:], arxc[:], arwc[:], op=OP.mult)
                sc_row = cp.tile([1, 2], F32)
                nc.vector.tensor_scalar_mul(sc_row[:, 0:1], hx[:], 1.0)
                nc.vector.tensor_scalar_mul(sc_row[:, 1:2], so0[:], S_OUT_C)
                psb2 = tpp.tile([128, 4, 128], F32, tag="t")
                nc.tensor.matmul(psb2[:, 0, 0:2], ones_row[:], sc_row[:],
                                 start=True, stop=True)
                nc.scalar.copy(scx[:], psb2[:, 0, 0:2])

                # ---- w stripes 2-7 -> gathers q1..q3 ----
                for i in range(2, N_LOC // 128):
                    w_stripe(i)
                    if i % 2 == 1:
                        gather_q(i // 2)

                # ---- wt quarter 0 DMA-in (from gathered DRAM) ----
                wts = []
                wt0 = wtp.tile([128, KSUB, N_CORES * QW], FP8, tag="wt")
                wts.append(wt0)
                for r in range(N_CORES):
                    blk = wT_all[0].ap()[r].rearrange("(ko p) n -> p ko n", p=128)
                    nc.sync.dma_start(wt0[:, :, r * QW:(r + 1) * QW], blk[:])

                def mm_round(qn, wt, mt):
                    rix = qn * (M_LOC // 128) + mt
                    pss = [mp.tile([128, N_TILE], F32, tag=f"ps{(rix * 4 + nb) % 6}",
                                   name=f"ps_{qn}_{mt}_{nb}")
                           for nb in range(4)]
                    for kp in range(KSUB // 2):
                        for nb in range(4):
                            mm = nc.tensor.matmul(
                                pss[nb][:],
                                xr[:, 2 * kp:2 * kp + 2,
                                   mt * 128:(mt + 1) * 128],
                                wt[:, 2 * kp:2 * kp + 2,
                                   nb * N_TILE:(nb + 1) * N_TILE],
                                start=(kp == 0), stop=(kp == KSUB // 2 - 1),
                                perf_mode=DR)
                            if ELIDE_LDW and nb > 0:
                                mm.ins.ldweights = False
                    m0 = mt * 128
                    for nb in range(4):
                        ep = epp.tile([128, N_TILE], F16, tag="ep")
                        nc.vector.scalar_tensor_tensor(
                            out=ep[:], in0=pss[nb][:], scalar=scx[:, 1:2],
                            in1=bias_t[:, qn * 2048 + nb * N_TILE:
                                       qn * 2048 + (nb + 1) * N_TILE],
                            op0=OP.mult, op1=OP.add)
                        for half in range(2):
                            r = 2 * nb + half
                            col = r * N_LOC + qn * QW
                            nc.sync.dma_start(
                                out[m0:m0 + 128, col:col + QW],
                                ep[:, half * QW:(half + 1) * QW])

                # ---- x re-read: quantize+transpose stripe i, then the
                # quarter-0 matmul round for the same m-tile (keeps PE hot) ----
                ev = 0
                for i in range(M_LOC // 128):
                    s = sp.tile([128, K], F32, tag="x", name=f"xq_in{i}")
                    nc.sync.dma_start(s[:], xs[i * 128:(i + 1) * 128, :])
                    xq = qp.tile([128, K], FP8, tag="xq", name=f"xq{i}")
                    nc.vector.tensor_scalar_mul(xq[:], s[:], scx[:, 0:1])
                    for kc4 in range(KSUB // 4):
                        t = tpp.tile([128, 4, 128], F32, tag="t")
                        for j in range(4):
                            kc = kc4 * 4 + j
                            nc.tensor.matmul(
                                t[:, j, :], xq[:, kc * 128:(kc + 1) * 128],
                                id8[:], start=True, stop=True)
                        dst = xr[:, kc4 * 4:kc4 * 4 + 4, i * 128:(i + 1) * 128]
                        if ev % 2 == 0:
                            nc.scalar.copy(dst, t[:])
                        else:
                            nc.vector.tensor_copy(dst, t[:])
                        ev += 1
                    mm_round(0, wt0, i)

                # ---- quarters 1..3 ----
                for qn in range(1, NQ):
                    wt = wtp.tile([128, KSUB, N_CORES * QW], FP8, tag="wt")
                    for r in range(N_CORES):
                        blk = wT_all[qn].ap()[r].rearrange(
                            "(ko p) n -> p ko n", p=128)
                        nc.sync.dma_start(wt[:, :, r * QW:(r + 1) * QW], blk[:])
                    for mt in range(M_LOC // 128):
                        mm_round(qn, wt, mt)
    nc.compile()
    return nc


_CACHE = {}


def _get(name, builder):
    if name not in _CACHE:
        _CACHE[name] = builder()
    return _CACHE[name]


def kernel(x: np.ndarray, w: np.ndarray, bias: np.ndarray) -> np.ndarray:
    global LAST_EXEC_NS
    LAST_EXEC_NS = []
    x = np.asarray(x)
    w = np.asarray(w)
    bias = np.asarray(bias)
    assert x.shape[-1] == K and w.shape == (N_FULL, K) and bias.shape == (N_FULL,)
    x2d = np.ascontiguousarray(x.reshape(-1, K).astype(np.float32, copy=False))
    assert x2d.shape[0] == M_FULL
    w = np.ascontiguousarray(w.astype(np.float32, copy=False))
    bias = bias.astype(np.float16, copy=False)

    cores = list(range(N_CORES))
    nc = _get("main", _build_main)

    # bias in gather order: chunk q of rank r holds w rows r*1024+q*256..+256
    bias_re = np.ascontiguousarray(
        bias.reshape(N_CORES, NQ, QW).transpose(1, 0, 2).reshape(1, N_FULL))

    ins = [
        {"xs": x2d[c * M_LOC:(c + 1) * M_LOC],
         "wl": w[c * N_LOC:(c + 1) * N_LOC],
         "bias_in": bias_re}
        for c in cores
    ]
    res = run_bass_kernel_spmd(nc, ins, core_ids=cores, trace=TRACE)
    if TRACE:
        LAST_EXEC_NS.append(res.exec_time_ns)

    out = np.concatenate([res.results[c]["out"] for c in cores], axis=0)
    return out.reshape(*x.shape[:-1], N_FULL)
